# revision 54
# baseline (speedup 1.0000x reference)
"""Multi-head causal attention on 8 TRN2 NeuronCores — one head per core.

Full inputs in, full output out. Per core (head h):
  Q^T/K^T = W^T x^T   (PE bf16, duplicated into both partition halves)
  S^T[j,i] = K_j . Q_i  (PE bf16, 128-deep duplicated contraction — keeps
                         the PE activity monitor at 2.4 GHz; the doubled
                         product folds into the exp scale)
  P^T = exp(S^T/16)     (ScalarE, 1024-wide calls, double-buffered PSUM)
  O^T[v,i] accum += V'[j,(v|1)]^T P^T[j,i]  (PE bf16; row 64 = sumexp)
  out[i,o] = (O^T[:,i]/sumexp_i)^T W_o      (PE + DVE row scale on evac)
Host sums the 8 per-head partial outputs (bf16 partials, f32 sum).
"""

import numpy as np
import ml_dtypes

import concourse.bass as bass
import concourse.mybir as mybir
import concourse.tile as tile
from concourse import bacc
from concourse.bass_utils import run_bass_kernel_spmd

BF16 = mybir.dt.bfloat16
F32 = mybir.dt.float32

S = 4096
D_IN = 512
D_K = 64
D_V = 64
D_OUT = 512
H = 8
NJT = S // 128   # 32 key tiles
NCH = S // 512   # 8 query chunks
NCK = D_IN // 128  # 4 contraction chunks for projections

_CACHE = {}


def _emit(nc, tc, ctx_pools):
    import contextlib

    xT_d = nc.dram_tensor("xT", [D_IN, S], BF16, kind="ExternalInput").ap()
    wq_d = nc.dram_tensor("wq", [D_IN, 128], BF16, kind="ExternalInput").ap()
    wk_d = nc.dram_tensor("wk", [D_IN, 128], BF16, kind="ExternalInput").ap()
    wv_d = nc.dram_tensor("wv", [D_IN, D_V], BF16, kind="ExternalInput").ap()
    wo_d = nc.dram_tensor("wo", [D_V, D_OUT], BF16, kind="ExternalInput").ap()
    mask_d = nc.dram_tensor("mask", [128, 128], BF16, kind="ExternalInput").ap()
    out_d = nc.dram_tensor("out", [S, D_OUT], BF16, kind="ExternalOutput").ap()

    Exp = mybir.ActivationFunctionType.Exp

    with contextlib.ExitStack() as ctx:
        const = ctx.enter_context(tc.tile_pool(name="const", bufs=1))
        persist = ctx.enter_context(tc.tile_pool(name="persist", bufs=1))
        small = ctx.enter_context(tc.tile_pool(name="small", bufs=3))
        outp = ctx.enter_context(tc.tile_pool(name="outp", bufs=3))

        # ---- constants ----
        # wq/wk arrive column-duplicated [512, 128] so the projection fills
        # both partition halves of Q^T/K^T (128-deep score contraction)
        wq_sb = const.tile([128, NCK * 128], BF16)
        wk_sb = const.tile([128, NCK * 128], BF16)
        wv_sb = const.tile([128, NCK * D_V], BF16)
        wo_sb = const.tile([D_V, D_OUT], BF16)
        mask_sb = const.tile([128, 128], BF16)
        for c in range(NCK):
            rows = slice(c * 128, (c + 1) * 128)
            nc.gpsimd.dma_start(out=wq_sb[:, c * 128:(c + 1) * 128], in_=wq_d[rows, :])
            nc.gpsimd.dma_start(out=wk_sb[:, c * 128:(c + 1) * 128], in_=wk_d[rows, :])
            nc.gpsimd.dma_start(out=wv_sb[:, c * D_V:(c + 1) * D_V], in_=wv_d[rows, :])
        nc.gpsimd.dma_start(out=wo_sb, in_=wo_d)
        nc.gpsimd.dma_start(out=mask_sb, in_=mask_d)

        # persistent activations
        qt = persist.tile([128, S], BF16)   # Q^T duplicated in both halves
        kt = persist.tile([128, S], BF16)   # K^T duplicated in both halves
        vp = persist.tile([128, NJT * 65], BF16)  # V' tiles [128, 65] per jt

        # ones column of every V' tile: strided [128, NJT] memset
        nc.vector.memset(
            vp.rearrange("p (j w) -> p j w", w=65)[:, :, 64], 1.0)

        # scratch used to warm up the PE clock gate / ScalarE act table
        # while the x^T DMAs are still in flight
        zt = const.tile([128, 256], BF16)
        nc.vector.memset(zt, 0.0)

        # ---- stage A: x^T streamed in 512-col blocks so the first
        # projections start after ~1.4 MB instead of the full 4 MB ----
        pt_pool = ctx.enter_context(tc.tile_pool(name="pt", bufs=1))
        xtp_ctx = contextlib.ExitStack()
        xtp = xtp_ctx.enter_context(tc.tile_pool(name="xt", bufs=1))
        xts = [xtp.tile([128, S], BF16, tag=f"xt{c}", name=f"xt{c}")
               for c in range(NCK)]
        for st in range(NCH):
            sl = bass.ts(st, 512)
            for c in range(NCK):
                nc.sync.dma_start(out=xts[c][:, sl],
                                  in_=xT_d[c * 128:(c + 1) * 128, sl])

        def proj(w_sb, wid, dest, st, pool, tag="psA"):
            sl = bass.ts(st, 512)
            ps = pool.tile([wid, 512], F32, tag=tag, name=f"ps{wid}_{st}")
            for c in range(NCK):
                nc.tensor.matmul(
                    ps,
                    lhsT=w_sb[:, c * wid:(c + 1) * wid],
                    rhs=xts[c][:, sl],
                    start=(c == 0),
                    stop=(c == NCK - 1),
                )
            nc.vector.tensor_copy(dest[:, sl], ps)

        with tc.tile_pool(name="psA", bufs=4, space="PSUM") as psA:
            # HAM warm-up: ~4us of back-to-back dummy matmuls so the real
            # projections below start at 2.4 GHz instead of 1.2; plus a tiny
            # exp to pull the ACT table load off the critical path
            hps = psA.tile([128, 256], F32, tag="heat")
            # >=3.4us of sustained activity flips the HAM clock gate to
            # 2.4 GHz; the burst also bridges the x^T DMA wait so the real
            # projections start warm with no re-throttling idle window
            for _ in range(24):
                nc.tensor.matmul(hps, lhsT=zt[:, 0:128], rhs=zt,
                                 start=True, stop=True)
            nc.scalar.activation(zt[0:128, 0:16], hps[:, 0:16], Exp,
                                 scale=0.0625)
            for st in range(2):
                proj(wq_sb, 128, qt, st, psA)
            proj(wk_sb, 128, kt, 0, psA)


        # ---- fused pass: S^T+exp, with O^T bursts filling PE exp-wait gaps ----
        from collections import deque
        pts = []
        pending = deque()  # closures, each emits one PE-side step of pass 2
        state = {"done": 0}
        TOT_GROUPS = sum(-(-(S - 128 * jt) // 1024) for jt in range(NJT))

        def drain(n):
            for _ in range(n):
                if not pending:
                    return
                pending.popleft()()
                state["done"] += 1

        def vp_direct(jt):
            # V' tile [128 keys, 64] = x_tile^T @ W_v — no transpose needed
            def go():
                pv = psFil.tile([128, 64], F32, tag="bank", name=f"pv{jt}")
                for c in range(NCK):
                    nc.tensor.matmul(
                        pv,
                        lhsT=xts[c][:, jt * 128:(jt + 1) * 128],
                        rhs=wv_sb[:, c * D_V:(c + 1) * D_V],
                        start=(c == 0),
                        stop=(c == NCK - 1),
                    )
                nc.vector.tensor_copy(vp[:, jt * 65:jt * 65 + 64], pv)
            return go

        def filler_proj(w_sb, wid, dest, st):
            def go():
                proj(w_sb, wid, dest, st, psFil, tag="bank")
            return go

        accs = {}
        nq = [0] * NCH

        def top_up(c, hi):
            hi = min(hi, 4 * c + 4)
            if hi > nq[c]:
                enqueue_ot(c, range(nq[c], hi))
                nq[c] = hi

        def enqueue_ot(c, j2s):
            if c not in accs:
                accs[c] = psOt.tile([65, 512], F32, tag="acc",
                                    name=f"acc{c}")
            acc = accs[c]
            jt_last = 4 * c + 3

            def ot_mm(j2):
                def go():
                    lo = max(c * 512, j2 * 128)
                    hi = (c + 1) * 512
                    nc.tensor.matmul(
                        acc[:, lo - c * 512:hi - c * 512],
                        lhsT=vp[:, j2 * 65:(j2 + 1) * 65],
                        rhs=pts[j2][:, lo - j2 * 128:hi - j2 * 128],
                        start=(j2 == 0),
                        stop=(j2 == jt_last),
                    )
                return go

            for j2 in j2s:
                pending.append(ot_mm(j2))

        def enqueue_fin(c, lo=0, hi=512, last=False):
            # finalize output rows [c*512+lo, c*512+hi); `last` routes work
            # onto ScalarE / the freed accumulator banks after exp is done
            acc = accs[c]
            ibs = list(range(lo // 128, hi // 128))

            def evac():
                ot_bf = small.tile([65, hi - lo], BF16, tag="otbf",
                                   name=f"otbf{c}_{lo}")
                if last:
                    nc.scalar.copy(ot_bf, acc[:, lo:hi])
                else:
                    nc.vector.tensor_copy(ot_bf, acc[:, lo:hi])
                se_bf = small.tile([128, 4], BF16, tag="se_bf",
                                   name=f"se{c}_{lo}")
                # sumexp row -> per-partition columns: tiny transposing
                # DMAs spread across queues so they run in parallel
                qs = (nc.scalar, nc.sync, nc.gpsimd, nc.sync) if last \
                    else (nc.sync, nc.gpsimd, nc.sync, nc.gpsimd)
                for k, ib in enumerate(ibs):
                    qs[k].dma_start(
                        out=se_bf[:, k:k + 1],
                        in_=ot_bf[64:65, ib * 128 - lo:(ib + 1) * 128 - lo],
                    )
                rcols = small.tile([128, 4], F32, tag="rcols",
                                   name=f"rc{c}_{lo}")
                nc.vector.reciprocal(rcols[:, 0:len(ibs)],
                                     se_bf[:, 0:len(ibs)])
                pos = {}

                def po_mm(k, ib):
                    # independent of rcols — keeps the PE busy while the
                    # sumexp reciprocal chain resolves. The final part uses
                    # the accumulator banks (free by then) instead of psFil.
                    def go():
                        pool, tg = (psOt, "acc") if last else (psFil, "bank")
                        po = pool.tile([128, 512], F32, tag=tg,
                                       name=f"po{c}_{ib}")
                        pos[k] = po
                        nc.tensor.matmul(
                            po,
                            lhsT=ot_bf[0:64, ib * 128 - lo:(ib + 1) * 128 - lo],
                            rhs=wo_sb,
                            start=True,
                            stop=True,
                        )
                    return go

                def po_scale(k, ib):
                    def go():
                        ob = outp.tile([128, 512], BF16, tag="ob")
                        if last and k == 0:
                            nc.scalar.mul(ob, pos[k], rcols[:, k:k + 1])
                        else:
                            nc.vector.tensor_scalar_mul(
                                ob, pos[k], rcols[:, k:k + 1])
                        nc.sync.dma_start(
                            out=out_d[c * 512 + ib * 128:
                                      c * 512 + (ib + 1) * 128, :],
                            in_=ob,
                        )
                    return go

                prev = None
                for k, ib in enumerate(ibs):
                    pending.append(po_mm(k, ib))
                    if prev is not None:
                        pending.append(po_scale(*prev))
                    prev = (k, ib)
                pending.append(po_scale(*prev))

            pending.append(evac)

        with tc.tile_pool(name="psB", bufs=2, space="PSUM") as psB, \
             tc.tile_pool(name="psFil", bufs=2, space="PSUM") as psFil, \
             tc.tile_pool(name="psOt", bufs=2, space="PSUM") as psOt:
            # filler work: remaining Q/K projections, V projections + V' tiles
            for st in range(2, NCH):
                pending.append(filler_proj(wq_sb, 128, qt, st))
            for st in range(1, NCH):
                pending.append(filler_proj(wk_sb, 128, kt, st))
            for j2 in range(NJT):
                pending.append(vp_direct(j2))
            N_HEAD = len(pending)  # closures that still read the x^T tiles
            gdone = 0
            for jt in range(NJT):
                i0 = jt * 128           # diagonal start
                pt = pt_pool.tile([128, S - i0], BF16, tag=f"pt{jt}")
                pts.append(pt)
                ktile = kt[:, jt * 128:(jt + 1) * 128]
                for g0 in range(i0, S, 1024):
                    ghi = min(g0 + 1024, S)
                    w = ghi - g0
                    ps = psB.tile([128, 1024], F32, tag="psB")
                    for lo in range(g0, ghi, 512):
                        hi = min(lo + 512, ghi)
                        nc.tensor.matmul(
                            ps[:, lo - g0:hi - g0],
                            lhsT=ktile,
                            rhs=qt[:, lo:hi],
                            start=True,
                            stop=True,
                        )
                    nc.scalar.activation(
                        pt[:, g0 - i0:ghi - i0],
                        ps[:, 0:w],
                        Exp,
                        scale=0.0625,  # 1/sqrt(64) / 2 (duplicated contraction)
                    )
                    gdone += 1
                    # pull PE filler work at a rate that empties the queue
                    # just as the exp stream ends
                    left = max(1, TOT_GROUPS - 6 - gdone)
                    drain(max(2, -(-len(pending) // left)))
                # causal mask on the diagonal 128x128 block
                nc.vector.tensor_mul(pt[:, 0:128], pt[:, 0:128], mask_sb)
                c = jt // 4
                if jt % 4 == 1:
                    top_up(c, jt + 1)
                    if jt == NJT - 3:
                        # rows 3584-3840 of the last chunk are complete
                        # (their columns see no further accumulation) —
                        # finalize them under the remaining exp stream
                        enqueue_fin(c, 0, 256, last=False)
                elif jt % 4 == 3:
                    top_up(c, jt + 1)
                    if c + 1 < NCH:
                        enqueue_fin(c)
                        # pre-enqueue the next chunk's already-available
                        # contributions so its O^T work spreads out early
                        top_up(c + 1, jt + 1)
                    else:
                        enqueue_fin(c, 256, 512, last=True)
                if jt == 19:
                    # open the last chunk extra-early (its accumulator takes
                    # the PSUM slot acc4 just vacated) so its O^T matmuls
                    # hide under the remaining exp stream instead of
                    # trailing it
                    top_up(NCH - 1, jt + 1)
                if jt == 9:
                    # finish everything that reads x^T, then free those tiles
                    # before the P^T pool reaches peak size; open chunk 3
                    # early (its slot was vacated back at jt=7) so the
                    # filler queue isn't dry right after this burst
                    drain(max(0, N_HEAD - state["done"]))
                    xtp_ctx.close()
                    top_up(3, jt + 1)
            while pending:
                drain(8)


def _build():
    if "nc" in _CACHE:
        return _CACHE["nc"]
    nc = bacc.Bacc("TRN2", target_bir_lowering=False, debug=False)
    with tile.TileContext(nc) as tc:
        _emit(nc, tc, None)
    nc.compile()
    _CACHE["nc"] = nc
    return nc


def build_in_maps(x, W_q, W_k, W_v, W_o):
    bf = ml_dtypes.bfloat16
    xT = np.ascontiguousarray(x.reshape(S, D_IN).T).astype(bf)
    mask = np.triu(np.ones((128, 128), np.float32)).astype(bf)
    in_maps = []
    for h in range(H):
        wq2 = np.concatenate([W_q[h], W_q[h]], axis=1)  # [512, 128]
        wk2 = np.concatenate([W_k[h], W_k[h]], axis=1)
        in_maps.append({
            "xT": xT,
            "wq": np.ascontiguousarray(wq2).astype(bf),
            "wk": np.ascontiguousarray(wk2).astype(bf),
            "wv": np.ascontiguousarray(W_v[h]).astype(bf),
            "wo": np.ascontiguousarray(W_o[h]).astype(bf),
            "mask": mask,
        })
    return in_maps


def kernel(x, W_q, W_k, W_v, W_o):
    nc = _build()
    in_maps = build_in_maps(x, W_q, W_k, W_v, W_o)
    res = run_bass_kernel_spmd(nc, in_maps, core_ids=list(range(H)))
    out = np.zeros((S, D_OUT), np.float32)
    for h in range(H):
        out += np.asarray(res.results[h]["out"], np.float32)
    return out[None]
